# revision 23
# baseline (speedup 1.0000x reference)
"""Trainium2 kernel for nn_MAg_90709709292194 (gnn_message_passing).

Computation: out = inputs @ ker_wt + bias, where ker_wt (8192x8192) holds the
`kernel` values scattered into the nonzero pattern of tile(adjacency, (4, 4))
in row-major nonzero order. The weight-matrix construction is build()-time
host work; the forward pass is the dense matmul on the NeuronCores.

Device strategy (8 cores, no collectives):
  - Output columns sharded: core k computes out[:, k*1024:(k+1)*1024].
  - The 16 MiB fp16 weight slice per core streams HBM->SBUF as the moving
    matmul operand; at ~360 GB/s/core this stream is the binding resource
    (~47 us), so everything else is arranged to hide under it:
      * the first weight groups issue on the scalar queue at t=0, the rest
        on the sync queue behind the X transpose (so the transpose isn't
        starved and both HWDGE queues keep the 16 DMA engines fed),
      * matmuls chase the weight stream group by group,
      * the band-reduce tail is pipelined in column halves.
  - X ships as fp16 (host cast, the same rounding the on-device cast DMA
    applied) and is transposed to K-major with one xbar DMA transpose;
    4-way PE column tiling (tile_position) packs four M=32 matmuls across
    the 128-wide array.
  - Per-band partial sums reduce via a block-identity stationary matmul;
    bias folds in as a K=1 matmul against a ones vector.

(An edge-wise sparse variant using gpsimd dma_gather was measured: SWDGE
descriptor generation costs ~8.4 ns/edge on the Q7 cluster and each 256B
gathered row occupies a DMA engine as long as a ~3.8KB dense packet, so at
~17 edges/column the sparse path's cost per column equals the dense path's.
Dense streaming with full overlap wins.)
"""

import numpy as np

N = 2048        # nodes
IN_CHAN = 4
CHANNELS = 4
B = 32          # batch
D = N * IN_CHAN     # 8192 contraction dim
DV = N * CHANNELS   # 8192 output dim
NCORES = 8
VS = DV // NCORES   # 1024 output columns per core
NT = D // 128       # 64 contraction tiles
NG = NT // 4        # 16 weight DMA groups (1 MiB each)

_PROGRAM_CACHE = {}


def build_program(debug=False):
    key = bool(debug)
    if key in _PROGRAM_CACHE:
        return _PROGRAM_CACHE[key]

    import concourse.bass as bass
    import concourse.bacc as bacc
    import concourse.mybir as mybir
    import concourse.tile as tile

    f32 = mybir.dt.float32
    f16 = mybir.dt.float16

    nc = bacc.Bacc(
        "TRN2", target_bir_lowering=False, debug=debug, num_devices=NCORES
    )
    xh = nc.dram_tensor("xh", [B, D], f16, kind="ExternalInput")
    wt = nc.dram_tensor("wt", [NG, 128, 4 * VS], f16, kind="ExternalInput")
    brow = nc.dram_tensor("brow", [1, VS], f16, kind="ExternalInput")
    red = nc.dram_tensor("red", [128, B], f16, kind="ExternalInput")
    out = nc.dram_tensor("out", [B, VS], f32, kind="ExternalOutput")

    with tile.TileContext(nc) as tc:
        with (
            tc.tile_pool(name="const", bufs=1) as const,
            tc.tile_pool(name="wpool", bufs=8) as wpool,
            tc.tile_pool(name="psum", bufs=1, space=bass.MemorySpace.PSUM) as psum,
        ):
            # X transpose first: xt[p, t, b] = X[b, t*128 + p] (xbar), racing
            # only the first weight groups for DMA engines.
            xt = const.tile([128, NT, B], f16)
            nc.sync.dma_start_transpose(out=xt[:], in_=xh[:])
            bs = const.tile([1, VS], f16)
            nc.sync.dma_start(out=bs[:], in_=brow[:])
            redsb = const.tile([128, B], f16)
            nc.sync.dma_start(out=redsb[:], in_=red[:])
            ones = const.tile([1, B], f16)
            nc.vector.memset(ones[:], 1.0)

            # Weight stream: the critical 16 MiB / ~47 us resource. First
            # groups issue immediately on the scalar queue; the rest issue
            # on the sync queue (after the xbar in its FIFO) so the
            # transpose isn't starved and both HWDGE queues feed engines.
            wsb = []
            for g in range(NG):
                w = wpool.tile([128, 4 * VS], f16, tag="wg", name=f"wg{g}")
                wsb.append(w)
            NEARLY = 8
            for g in range(NEARLY):
                nc.scalar.dma_start(out=wsb[g][:], in_=wt[g])
            for g in range(NEARLY, NG):
                nc.sync.dma_start(out=wsb[g][:], in_=wt[g])

            # 4-way PE column tiling: u-tile t of each group lands its M=32
            # output on partitions [32t, 32t+32); partials reduced across
            # bands by a block-identity matmul afterwards.
            acc = psum.tile([128, VS], f32)
            for g in range(NG):
                for t in range(4):
                    ut = g * 4 + t
                    for h in range(2):
                        nc.tensor.matmul(
                            acc[32 * t : 32 * (t + 1), h * 512 : (h + 1) * 512],
                            xt[:, ut, :],
                            wsb[g][:, t * VS + h * 512 : t * VS + (h + 1) * 512],
                            start=(g == 0),
                            stop=(g == NG - 1),
                            tile_position=(0, 32 * t),
                            skip_group_check=True,
                        )

            # partial reduce: bias folded via a K=1 ones matmul, then
            # out[b] = sum_j ph[32j + b] via a block-identity stationary.
            # Pipelined in column halves so DVE copies overlap PE reduces.
            ph = const.tile([128, VS], f16)
            acc2 = psum.tile([B, VS], f32, tag="acc2")
            osb = const.tile([B, VS], f32)
            for h in range(2):
                sl = slice(h * 512, (h + 1) * 512)
                nc.vector.tensor_copy(ph[:, sl], acc[:, sl])
                nc.tensor.matmul(
                    acc2[:, sl], redsb[:], ph[:, sl], start=True, stop=False,
                    skip_group_check=True,
                )
                nc.tensor.matmul(
                    acc2[:, sl], ones[:], bs[:, sl], start=False, stop=True,
                    skip_group_check=True,
                )
            for h in range(2):
                sl = slice(h * 512, (h + 1) * 512)
                nc.vector.tensor_copy(osb[:, sl], acc2[:, sl])
                nc.sync.dma_start(out=out[:, sl], in_=osb[:, sl])

    nc.compile()
    _PROGRAM_CACHE[key] = nc
    return nc


def pack_inputs(inputs, adjacency, kernel, bias):
    """Host-side build()-time weight construction + per-core sharding.
    X ships as fp16 (same rounding the on-device cast DMA applied)."""
    Xh = np.ascontiguousarray(np.asarray(inputs).astype(np.float16))
    A = np.asarray(adjacency, dtype=np.float32)
    kern = np.asarray(kernel, dtype=np.float32)
    b = np.asarray(bias, dtype=np.float32)

    rows, cols = np.nonzero(A)
    nnz = rows.shape[0]
    rnnz = np.bincount(rows, minlength=N).astype(np.int64)
    prefix = np.concatenate([[0], np.cumsum(rnnz)[:-1]])
    k_in_row = np.arange(nnz, dtype=np.int64) - prefix[rows]
    base_r = 4 * prefix[rows]
    rn = rnnz[rows]

    W = np.zeros((D, DV), np.float16)
    for c_in in range(IN_CHAN):
        for c_out in range(CHANNELS):
            idx = 4 * nnz * c_in + base_r + c_out * rn + k_in_row
            W[c_in * N + rows, c_out * N + cols] = kern[idx]
    bh = b.astype(np.float16)
    red = np.zeros((128, B), np.float16)
    for j in range(128 // B):
        red[j * B + np.arange(B), np.arange(B)] = 1.0

    in_maps = []
    for k in range(NCORES):
        ws = (
            W[:, k * VS : (k + 1) * VS]
            .reshape(NG, 4, 128, VS)
            .transpose(0, 2, 1, 3)
            .reshape(NG, 128, 4 * VS)
        )
        in_maps.append(
            {
                "xh": Xh,
                "wt": np.ascontiguousarray(ws),
                "brow": np.ascontiguousarray(bh[None, k * VS : (k + 1) * VS]),
                "red": red,
            }
        )
    return in_maps


def unpack_output(results):
    return np.concatenate([results[k]["out"] for k in range(NCORES)], axis=1)


def run(in_maps, trace=False, **kwargs):
    from concourse.bass_utils import run_bass_kernel_spmd

    nc = build_program(debug=False)
    res = run_bass_kernel_spmd(
        nc, in_maps, core_ids=list(range(NCORES)), trace=trace, **kwargs
    )
    return res


def run_full(packed, trace=False, **kwargs):
    res = run(packed, trace=trace, **kwargs)
    return unpack_output(res.results), res


def kernel(inputs, adjacency, kernel, bias):
    out, _ = run_full(pack_inputs(inputs, adjacency, kernel, bias))
    return out
